# revision 86
# baseline (speedup 1.0000x reference)
"""AttnBlock (GroupNorm -> QKV 1x1 convs -> full NxN attention -> out proj + residual)
for B=8, C=512, H=W=64 on 8 Trainium2 NeuronCores.

Sharding: data-parallel over batch, one sample per core. Each core holds the
full (host-pretransposed, bf16-cast) [C,C] conv weights and processes its
sample's [C, N=4096] activations entirely on-chip.

Per-core kernel outline (big matmuls fp8e4m3 DoubleRow with fp32 PSUM accumulate):
  1. GroupNorm(32 groups of 16 channels): bn_stats/bn_aggr per channel; since
     a group never straddles a 128-channel chunk, the group reduce/broadcast
     (tiny indicator matmuls on PE) and the ACT affine pass x*scl+bia -> h
     (bf16) pipeline per chunk.
  2. q = wq@h+bq, k = wk@h+bk as [C,N]; v computed directly transposed as
     vT = (h^T)@wv^T + bv -> [N,C] (so the attention V-matmul needs no
     transposes).
  3. Attention per 512-wide query block: S^T[j,i] = sum_c k[c,j] q[c,i]
     (accumulate 4 chunk matmuls in PSUM), P^T = exp(S^T/sqrt(C)) evicted by
     ScalarE to bf16 SBUF; softmax denominators by DVE-accumulating the 32
     P^T chunks then one ones-matmul partition reduce; O = V@P^T accumulated
     over the 32 j-chunks; normalize by broadcasting 1/denom across
     partitions with a K=1 matmul.
  4. out = wo@Ohat + bo + x (bias+residual fused in one DVE eviction).
"""

import numpy as np
import ml_dtypes

B = 8
C = 512
H = 64
W = 64
N = H * W            # 4096
P = 128
KC = C // P          # 4 channel chunks
NB = N // 512        # 8 blocks of 512
NT = N // P          # 32 tiles of 128
NGL = P // 16        # 8 groups per 128-channel chunk (group size 16)
EPS = 1e-6
SCALE = float(C) ** -0.5

_CACHE = {}

# Attention matmuls (S^T = K^T@Q and O = V@P^T) in fp8e4m3 with DoubleRow
# (K=256 per matmul, 2x PE throughput). exp outputs are pre-scaled by
# exp(PBIAS) to stay under e4m3's +-240 range; the softmax denominator gets
# the same factor, so normalization cancels it exactly.
FP8_ATTN = True
PBIAS = -2.772588722239781  # ln(1/16)
# QKV convs in fp8 DoubleRow, consuming an fp8 copy of x (GroupNorm stats also
# come from it: quantization noise averages out over the 65536-element groups).
FP8_CONV = True
# Output projection in fp8 DoubleRow.
FP8_WO = True
# Power-of-2 gains that keep fp8 operands out of the subnormal range (all are
# compensated exactly elsewhere):
#   GW: q/k/v weights scaled on device (w*scl*GW); q,k stored *GW so scores
#       come out *GW^2 -> exp scale divides by GW^2. v stored *GW -> folded
#       into the reciprocal row.
#   OH_GAIN: Ohat stored *OH_GAIN (fp8) -> final eviction divides it out.
#   GO: wo scaled *GO on the host -> final eviction divides it out.
GW = 8.0
OH_GAIN = 16.0
GO = 8.0
# ScalarE's exp stream paces the attention phase; offload these j-tiles (per
# 512-query block) to VectorE via the Schraudolph exp2 bit-trick:
#   exp(y) ~= bitcast_f32(int32((y*log2e + 127 - 0.043) * 2^23))
# One fused DVE multiply-add (fp32->int32) + one DVE copy (bitcast->fp8);
# ~3% relative error on ~12% of the attention weights.
EXP2_JTS = (11, 15, 19, 23, 27)  # mid-late: DVE runs the prior block's tail early on
LOG2E = 1.4426950408889634
EXP2_C = 0.04367744890362246


def _build_nc():
    import concourse.bass as bass
    import concourse.mybir as mybir
    import concourse.tile as tile
    from concourse import bacc

    fp32 = mybir.dt.float32
    bf16 = mybir.dt.bfloat16
    fp8 = mybir.dt.float8e4
    adt = fp8 if FP8_ATTN else bf16
    DR = mybir.MatmulPerfMode.DoubleRow
    AF = mybir.ActivationFunctionType
    Alu = mybir.AluOpType

    nc = bacc.Bacc("TRN2", target_bir_lowering=False, debug=False)

    cdt = fp8 if FP8_CONV else bf16  # conv activation dtype
    x_d = nc.dram_tensor("x", [C, N], fp32, kind="ExternalInput")
    xb_d = nc.dram_tensor("xb", [C, N], cdt, kind="ExternalInput")
    # Pretransposed conv weights: q, k, v in one tensor; wo separate (it is
    # not needed until phase B, so its DMA goes last).
    ww_d = nc.dram_tensor("ww", [3, C, C], bf16, kind="ExternalInput")
    wo_d = nc.dram_tensor("wod", [C, C], fp8 if FP8_WO else bf16, kind="ExternalInput")
    # Per-channel params packed: gn_scale, gn_bias, bq, bk, bo.
    prm_d = nc.dram_tensor("prm", [5, C], fp32, kind="ExternalInput")
    bv_d = nc.dram_tensor("bv", [C], fp32, kind="ExternalInput")
    ind_d = nc.dram_tensor("ind", [P, NGL], fp32, kind="ExternalInput")
    indT_d = nc.dram_tensor("indT", [NGL, P], fp32, kind="ExternalInput")
    out_d = nc.dram_tensor("out", [C, N], fp32, kind="ExternalOutput")

    with tile.TileContext(nc) as tc:
        with tc.tile_pool(name="persist", bufs=1) as pp:
            # ---------------- Phase A: GroupNorm + QKV convs ----------------
            with tc.tile_pool(name="pa", bufs=2) as pa, \
                 tc.tile_pool(name="pa_ps", bufs=1, space="PSUM") as pa_ps, \
                 tc.tile_pool(name="pa1", bufs=1) as pa1:
                # x (bf16) first: its DMAs gate the whole startup critical path.
                # Half-chunk transfers so bn_stats can start sooner.
                xbs = pa1.tile([P, KC, N], cdt, tag="xbs")
                nc.sync.dma_start(
                    out=xbs[:, 0, 0:2048], in_=xb_d.ap()[0:P, 0:2048]
                )
                # Small tensors needed by the early stats chain go ahead of the
                # bulk transfers.
                ind = pp.tile([P, NGL], fp32, tag="ind")
                nc.sync.dma_start(out=ind, in_=ind_d.ap())
                indT = pp.tile([NGL, P], fp32, tag="indT")
                nc.sync.dma_start(out=indT, in_=indT_d.ap())
                # Per-channel params in [p, kc] layout (channel = kc*128 + p).
                prm = pp.tile([P, 5, KC], fp32, tag="prm")
                nc.sync.dma_start(
                    out=prm, in_=prm_d.ap().rearrange("s (k p) -> p s k", p=P)
                )
                gns, gnb, bqs, bks, bos = (prm[:, i] for i in range(5))
                nc.sync.dma_start(
                    out=xbs[:, 0, 2048:4096], in_=xb_d.ap()[0:P, 2048:4096]
                )
                # Weights: [c', kc-of-c', c], contraction dim c' on partitions.
                ww = pp.tile([P, 3, KC, C], bf16, tag="ww")
                nc.sync.dma_start(
                    out=ww, in_=ww_d.ap().rearrange("w (kc p) c -> p w kc c", p=P)
                )
                wqt, wkt, wvt = (ww[:, i] for i in range(3))
                for kc in range(1, KC):
                    for hh in range(2):
                        nc.sync.dma_start(
                            out=xbs[:, kc, hh * 2048:(hh + 1) * 2048],
                            in_=xb_d.ap()[kc * P:(kc + 1) * P, hh * 2048:(hh + 1) * 2048],
                        )
                wot = pp.tile([P, KC, C], fp8 if FP8_WO else bf16, tag="wot")
                nc.sync.dma_start(
                    out=wot, in_=wo_d.ap().rearrange("(kc p) c -> p kc c", p=P)
                )
                bvr = pp.tile([1, C], fp32, tag="bvr")
                nc.sync.dma_start(
                    out=bvr, in_=bass.AP(tensor=bv_d, offset=0, ap=[[0, 1], [1, C]])
                )

                ones_f = pp.tile([P, 1], fp32, tag="ones_f")
                nc.vector.memset(ones_f, 1.0)
                ones_r = pp.tile([1, P], bf16, tag="ones_r")
                nc.vector.memset(ones_r, 1.0)
                eps8 = pp.tile([NGL, 1], fp32, tag="eps8")
                nc.vector.memset(eps8, EPS)
                pbias = pp.tile([P, 1], fp32, tag="pbias")
                nc.vector.memset(pbias, PBIAS)
                # DoubleRow lhsT needs the pair stride to be 16B-aligned, so
                # pad the ones column out to 16 elements and slice.
                ones8w = pp.tile([P, 2, 16], fp8, tag="ones8")
                nc.vector.memset(ones8w, 1.0)
                ones8 = ones8w[:, :, 0:1]

                qsb = pp.tile([P, KC, N], adt, tag="qsb")
                ksb = pp.tile([P, KC, N], adt, tag="ksb")
                vt = pp.tile([P, NT, C], adt, tag="vt")

                scl = pa1.tile([P, KC], fp32, tag="scl")
                sclG = pa1.tile([P, KC], fp32, tag="sclG")
                bia = pa1.tile([P, KC], fp32, tag="bia")
                bia_bf = pa1.tile([P, KC], bf16, tag="bia_bf")
                # GroupNorm affine folded into the conv weights:
                #   conv(w, h) = conv(w*scl, x) + w@bia, so scale each weight
                #   column c' by scl[c'] (per-partition multiply on the
                #   pretransposed weights) and fix the biases with tiny
                #   w@bia matmuls. x never needs normalizing, so PE can start
                #   on partial conv accumulations while later chunks' stats
                #   are still in flight.
                ws = pa1.tile([P, 3, KC, C], cdt, tag="ws")

                # Groups (16 channels) never straddle a 128-channel chunk, so
                # the whole stats -> scale/bias -> h chain pipelines per chunk.
                for kc in range(KC):
                    st = pa.tile([P, 8, 6], fp32, tag="bnst")
                    for sg in range(8):
                        nc.vector.bn_stats(
                            out=st[:, sg, :], in_=xbs[:, kc, sg * 512:(sg + 1) * 512]
                        )
                    stats = pa.tile([P, 3], fp32, tag="stats")  # mean, var, mean^2
                    nc.vector.bn_aggr(out=stats[:, 0:2], in_=st)
                    nc.vector.tensor_mul(
                        out=stats[:, 2:3], in0=stats[:, 0:1], in1=stats[:, 0:1]
                    )
                    # Sum stats over the 16 channels of each group.
                    ps_st = pa_ps.tile([NGL, 3], fp32, tag="ps_st")
                    nc.tensor.matmul(ps_st, lhsT=ind, rhs=stats, start=True, stop=True)
                    st_s = pa.tile([NGL, 3], fp32, tag="st_s")
                    nc.vector.tensor_copy(out=st_s, in_=ps_st)

                    bcin = pa.tile([NGL, 2], fp32, tag="bcin")  # [:,0]=gmean [:,1]=rstd
                    nc.vector.tensor_scalar_mul(
                        out=bcin[:, 0:1], in0=st_s[:, 0:1], scalar1=1.0 / 16
                    )
                    gv = pa.tile([NGL, 1], fp32, tag="gv")
                    nc.vector.tensor_add(out=gv, in0=st_s[:, 1:2], in1=st_s[:, 2:3])
                    nc.vector.tensor_scalar_mul(out=gv, in0=gv, scalar1=1.0 / 16)
                    gm2 = pa.tile([NGL, 1], fp32, tag="gm2")
                    nc.vector.tensor_mul(out=gm2, in0=bcin[:, 0:1], in1=bcin[:, 0:1])
                    nc.vector.tensor_sub(out=gv, in0=gv, in1=gm2)  # var
                    nc.scalar.activation(out=gv, in_=gv, func=AF.Sqrt, bias=eps8)
                    nc.vector.reciprocal(out=bcin[:, 1:2], in_=gv)

                    # Broadcast group mean/rstd back to the chunk's channels.
                    ps_bc = pa_ps.tile([P, 2], fp32, tag="ps_bc")
                    nc.tensor.matmul(ps_bc, lhsT=indT, rhs=bcin, start=True, stop=True)
                    nc.vector.tensor_mul(
                        out=scl[:, kc:kc + 1], in0=ps_bc[:, 1:2], in1=gns[:, kc:kc + 1]
                    )
                    nc.vector.tensor_mul(
                        out=bia[:, kc:kc + 1], in0=ps_bc[:, 0:1], in1=scl[:, kc:kc + 1]
                    )
                    nc.vector.tensor_sub(
                        out=bia[:, kc:kc + 1], in0=gnb[:, kc:kc + 1], in1=bia[:, kc:kc + 1]
                    )
                    nc.vector.tensor_copy(
                        out=bia_bf[:, kc:kc + 1], in_=bia[:, kc:kc + 1]
                    )
                    # Weight scaling as a scaled ACT copy (keeps DVE free for
                    # the stats + vT eviction stream).
                    nc.vector.tensor_scalar_mul(
                        out=sclG[:, kc:kc + 1], in0=scl[:, kc:kc + 1],
                        scalar1=GW if FP8_CONV else 1.0,
                    )
                    for w in range(3):
                        nc.scalar.activation(
                            out=ws[:, w, kc, :], in_=ww[:, w, kc, :],
                            func=AF.Copy, scale=sclG[:, kc:kc + 1],
                        )

                # Folded conv biases: qb/kb[c] = w@bia + b  (per-partition in
                # the [C, N] output layout).
                qkb = pa1.tile([P, 2, KC], fp32, tag="qkb")
                for w, bsl in ((0, bqs), (1, bks)):
                    for ct in range(KC):
                        pqb = pa_ps.tile([P, 1], fp32, tag="qb", bufs=1)
                        for kc in range(KC):
                            nc.tensor.matmul(
                                pqb,
                                lhsT=ww[:, w, kc, ct * P:(ct + 1) * P],
                                rhs=bia_bf[:, kc:kc + 1],
                                start=(kc == 0), stop=(kc == KC - 1),
                            )
                        nc.vector.tensor_scalar(
                            out=qkb[:, w, ct:ct + 1], in0=pqb,
                            scalar1=bsl[:, ct:ct + 1],
                            scalar2=GW if FP8_CONV else 1.0,
                            op0=Alu.add, op1=Alu.mult,
                        )
                # vT bias row: vb[c] = bia@wvT + bv, broadcast to all partitions.
                pvb = pa_ps.tile([1, C], fp32, tag="vb", bufs=1)
                for kc in range(KC):
                    nc.tensor.matmul(
                        pvb, lhsT=bia_bf[:, kc:kc + 1], rhs=wvt[:, kc, :],
                        start=(kc == 0), stop=(kc == KC - 1),
                    )
                vbr = pa1.tile([1, C], fp32, tag="vbr")
                nc.vector.tensor_add(out=vbr, in0=pvb, in1=bvr)
                vbr_bf = pa1.tile([1, C], bf16, tag="vbr_bf")
                nc.vector.tensor_scalar_mul(
                    out=vbr_bf, in0=vbr, scalar1=GW if FP8_CONV else 1.0
                )
                pbv = pa_ps.tile([P, C], fp32, tag="convp", bufs=4)
                nc.tensor.matmul(pbv, lhsT=ones_r, rhs=vbr_bf, start=True, stop=True)
                bvb = pa1.tile([P, C], fp32, tag="bvb")
                nc.scalar.copy(out=bvb, in_=pbv)

                # q, k convs: [C, N] layout, consuming raw x.
                def conv_mms(psum, w_lhsT, w_rhs):
                    if FP8_CONV:
                        for k2 in range(KC // 2):
                            kk = slice(2 * k2, 2 * k2 + 2)
                            nc.tensor.matmul(
                                psum, lhsT=w_lhsT(kk), rhs=w_rhs(kk),
                                start=(k2 == 0), stop=(k2 == KC // 2 - 1),
                                perf_mode=DR,
                            )
                    else:
                        for kc in range(KC):
                            kk = slice(kc, kc + 1)
                            nc.tensor.matmul(
                                psum,
                                lhsT=w_lhsT(kk)[:, 0], rhs=w_rhs(kk)[:, 0],
                                start=(kc == 0), stop=(kc == KC - 1),
                            )

                # Interleave q/k/vT convs (nb-outer) so the eviction work
                # streams on ACT and DVE concurrently; k evictions alternate
                # between the two to balance their loads.
                for nb in range(NB):
                    for ct in range(KC):
                        pq = pa_ps.tile([P, 512], fp32, tag="convp", bufs=4)
                        conv_mms(
                            pq,
                            lambda kk, ct=ct: ws[:, 0, kk, ct * P:(ct + 1) * P],
                            lambda kk, nb=nb: xbs[:, kk, nb * 512:(nb + 1) * 512],
                        )
                        nc.scalar.activation(
                            out=qsb[:, ct, nb * 512:(nb + 1) * 512], in_=pq,
                            func=AF.Identity, bias=qkb[:, 0, ct:ct + 1],
                        )
                        pk = pa_ps.tile([P, 512], fp32, tag="convp", bufs=4)
                        conv_mms(
                            pk,
                            lambda kk, ct=ct: ws[:, 1, kk, ct * P:(ct + 1) * P],
                            lambda kk, nb=nb: xbs[:, kk, nb * 512:(nb + 1) * 512],
                        )
                        if ct % 4 != 3:
                            nc.scalar.activation(
                                out=ksb[:, ct, nb * 512:(nb + 1) * 512], in_=pk,
                                func=AF.Identity, bias=qkb[:, 1, ct:ct + 1],
                            )
                        else:
                            nc.vector.tensor_scalar_add(
                                out=ksb[:, ct, nb * 512:(nb + 1) * 512], in0=pk,
                                scalar1=qkb[:, 1, ct:ct + 1],
                            )
                    # vT conv: vT[n, c] = sum_c' x[c', n] ws_v[c', c] + vb[c].
                    for jt in range(4 * nb, 4 * nb + 4):
                        pv = pa_ps.tile([P, 512], fp32, tag="convp", bufs=4)
                        conv_mms(
                            pv,
                            lambda kk, jt=jt: xbs[:, kk, jt * P:(jt + 1) * P],
                            lambda kk: ws[:, 2, kk, :],
                        )
                        nc.vector.tensor_add(out=vt[:, jt, :], in0=pv, in1=bvb)

            # ---------------- Phase B: attention + output ----------------
            # Software-pipelined across query blocks: block ib-1's normalize /
            # projection tail is emitted inside block ib's score stream (the
            # broadcast matmul at jt==2, the projections at jt==8) so the
            # in-order PE always has S-matmuls to run while DVE works through
            # the reciprocal/normalize chain.
            with tc.tile_pool(name="pb", bufs=1) as pb, \
                 tc.tile_pool(name="pb_ps", bufs=1, space="PSUM") as pb_ps:
                escale = SCALE / (GW * GW) if FP8_CONV else SCALE
                rgain = (OH_GAIN if FP8_WO else 1.0) / (GW if FP8_CONV else 1.0)

                def tail1(ib, po, pd):
                    # 1/denom -> broadcast across partitions -> Ohat.
                    rc = pb.tile([1, 512], fp32, tag="rc", bufs=2)
                    nc.vector.reciprocal(out=rc, in_=pd)
                    rcb = pb.tile([1, 512], bf16, tag="rcb", bufs=2)
                    nc.vector.tensor_scalar_mul(out=rcb, in0=rc, scalar1=rgain)
                    pw = pb_ps.tile([P, 512], fp32, tag="w", bufs=1, name="pbc")
                    nc.tensor.matmul(pw, lhsT=ones_r, rhs=rcb, start=True, stop=True)
                    bcs = pb.tile([P, 512], fp32, tag="bcs", bufs=2)
                    nc.vector.tensor_copy(out=bcs, in_=pw)
                    oh = pb.tile([P, KC, 512], fp8 if FP8_WO else bf16, tag="oh", bufs=2)
                    for ct in range(KC):
                        nc.vector.tensor_mul(out=oh[:, ct, :], in0=po[ct], in1=bcs)
                    return oh

                def tail2(ib, oh, last=False):
                    # Output projection + bias + residual + store. The final
                    # block's projections use the freed 4-bank "o" tag so the
                    # four c-tiles retire in parallel instead of serializing
                    # on the single shared "w" bank.
                    for ct in range(KC):
                        if last:
                            pf = pb_ps.tile([P, 512], fp32, tag="o", bufs=4, name="pf")
                        else:
                            pf = pb_ps.tile([P, 512], fp32, tag="w", bufs=1, name="pf")
                        if FP8_WO:
                            for k2 in range(KC // 2):
                                nc.tensor.matmul(
                                    pf,
                                    lhsT=wot[:, 2 * k2:2 * k2 + 2, ct * P:(ct + 1) * P],
                                    rhs=oh[:, 2 * k2:2 * k2 + 2, :],
                                    start=(k2 == 0), stop=(k2 == KC // 2 - 1),
                                    perf_mode=DR,
                                )
                        else:
                            for kc in range(KC):
                                nc.tensor.matmul(
                                    pf,
                                    lhsT=wot[:, kc, ct * P:(ct + 1) * P],
                                    rhs=oh[:, kc, :],
                                    start=(kc == 0), stop=(kc == KC - 1),
                                )
                        xr = pb.tile([P, 512], fp32, tag="xr", bufs=3)
                        nc.sync.dma_start(
                            out=xr,
                            in_=x_d.ap()[ct * P:(ct + 1) * P, ib * 512:(ib + 1) * 512],
                        )
                        ob = pb.tile([P, 512], fp32, tag="ob", bufs=3)
                        if FP8_WO:
                            # x + bo staged on the idle GpSimd engine, then the
                            # eviction divides out OH_GAIN*GO.
                            xrb = pb.tile([P, 512], fp32, tag="xrb", bufs=3)
                            nc.gpsimd.tensor_scalar_add(
                                out=xrb, in0=xr, scalar1=bos[:, ct:ct + 1]
                            )
                            nc.vector.scalar_tensor_tensor(
                                out=ob, in0=pf, scalar=1.0 / (OH_GAIN * GO), in1=xrb,
                                op0=Alu.mult, op1=Alu.add,
                            )
                        else:
                            nc.vector.scalar_tensor_tensor(
                                out=ob, in0=pf, scalar=bos[:, ct:ct + 1], in1=xr,
                                op0=Alu.add, op1=Alu.add,
                            )
                        nc.sync.dma_start(
                            out=out_d.ap()[ct * P:(ct + 1) * P, ib * 512:(ib + 1) * 512],
                            in_=ob,
                        )

                pending = None  # (ib, po, pd) awaiting its tail
                pending_oh = None
                PEEL = 5

                def emit_sjt(ib, jt, pt):
                    ps = pb_ps.tile([P, 512], fp32, tag="s", bufs=3)
                    for k2 in range(KC // 2):
                        nc.tensor.matmul(
                            ps,
                            lhsT=ksb[:, 2 * k2:2 * k2 + 2, jt * P:(jt + 1) * P],
                            rhs=qsb[:, 2 * k2:2 * k2 + 2, ib * 512:(ib + 1) * 512],
                            start=(k2 == 0), stop=(k2 == KC // 2 - 1),
                            perf_mode=DR,
                        )
                    if jt in EXP2_JTS:
                        zi = pb.tile([P, 512], mybir.dt.int32, tag="zi", bufs=2)
                        nc.vector.tensor_scalar(
                            out=zi, in0=ps,
                            scalar1=escale * LOG2E * 8388608.0,
                            scalar2=(PBIAS * LOG2E + 127.0 - EXP2_C) * 8388608.0,
                            op0=Alu.mult, op1=Alu.add,
                        )
                        nc.vector.tensor_copy(out=pt[:, jt, :], in_=zi.bitcast(fp32))
                    else:
                        nc.scalar.activation(
                            out=pt[:, jt, :], in_=ps, func=AF.Exp,
                            scale=escale, bias=pbias,
                        )

                def new_pt():
                    pt = pb.tile([P, NT, 512], adt, tag="pt", bufs=2)
                    return pt

                # Peel the first PEEL score-tiles of each block ahead of the
                # previous block's O-accumulation so ScalarE keeps streaming
                # exp work across block boundaries.
                pt_cur = new_pt()
                for jt in range(PEEL):
                    emit_sjt(0, jt, pt_cur)
                for ib in range(NB):
                    for jt in range(PEEL, NT):
                        emit_sjt(ib, jt, pt_cur)
                        if jt == PEEL + 1 and pending is not None:
                            pending_oh = tail1(*pending)
                        if jt == 9 and pending is not None:
                            tail2(pending[0], pending_oh)
                            pending = pending_oh = None
                    pt_prev = pt_cur
                    if ib + 1 < NB:
                        pt_cur = new_pt()
                        for jt in range(PEEL):
                            emit_sjt(ib + 1, jt, pt_cur)
                    # Softmax denominators: column sums of P^T.
                    pd = pb_ps.tile([1, 512], fp32, tag="w", bufs=1, name="pd")
                    for jt2 in range(NT // 2):
                        nc.tensor.matmul(
                            pd, lhsT=ones8, rhs=pt_prev[:, 2 * jt2:2 * jt2 + 2, :],
                            start=(jt2 == 0), stop=(jt2 == NT // 2 - 1),
                            perf_mode=DR,
                        )
                    # O = V @ P^T, accumulated over j chunks.
                    po = [
                        pb_ps.tile([P, 512], fp32, tag="o", bufs=4, name=f"po{ct}")
                        for ct in range(KC)
                    ]
                    for jt2 in range(NT // 2):
                        for ct in range(KC):
                            nc.tensor.matmul(
                                po[ct],
                                lhsT=vt[:, 2 * jt2:2 * jt2 + 2, ct * P:(ct + 1) * P],
                                rhs=pt_prev[:, 2 * jt2:2 * jt2 + 2, :],
                                start=(jt2 == 0), stop=(jt2 == NT // 2 - 1),
                                perf_mode=DR,
                            )
                    pending = (ib, po, pd)
                # Last block's tail.
                oh = tail1(*pending)
                tail2(pending[0], oh, last=True)

    nc.compile()
    return nc


def _get_nc():
    if "nc" not in _CACHE:
        _CACHE["nc"] = _build_nc()
    return _CACHE["nc"]


def _indicator():
    ind = np.zeros((P, NGL), np.float32)
    for g in range(NGL):
        ind[g * 16:(g + 1) * 16, g] = 1.0
    return ind


def host_inputs(x, gn_scale, gn_bias, wq, bq, wk, bk, wv, bv, wo, bo):
    """Host-side reformatting: per-core input maps (x is [B?, C, H*W] or [C, H*W])."""
    bf = ml_dtypes.bfloat16
    f8 = ml_dtypes.float8_e4m3
    xf = np.ascontiguousarray(np.asarray(x, dtype=np.float32).reshape(-1, C, N))
    xbf = xf.astype(f8 if FP8_CONV else bf)
    ww = np.stack(
        [np.asarray(w, np.float32).T for w in (wq, wk, wv)], axis=0
    ).astype(bf)
    wod = np.ascontiguousarray(np.asarray(wo, np.float32).T * (GO if FP8_WO else 1.0)
                               ).astype(f8 if FP8_WO else bf)
    prm = np.stack(
        [np.asarray(v, np.float32) for v in (gn_scale, gn_bias, bq, bk, bo)], axis=0
    )
    common = {
        "ww": np.ascontiguousarray(ww),
        "wod": wod,
        "prm": np.ascontiguousarray(prm),
        "bv": np.asarray(bv, np.float32),
        "ind": _indicator(),
        "indT": np.ascontiguousarray(_indicator().T),
    }
    return [dict(common, x=xf[i], xb=xbf[i]) for i in range(xf.shape[0])]


def kernel(x, gn_scale, gn_bias, wq, bq, wk, bk, wv, bv, wo, bo):
    from concourse.bass_utils import run_bass_kernel_spmd

    nc = _get_nc()
    in_maps = host_inputs(x, gn_scale, gn_bias, wq, bq, wk, bk, wv, bv, wo, bo)
    res = run_bass_kernel_spmd(nc, in_maps, core_ids=list(range(B)))
    out = np.stack([res.results[i]["out"] for i in range(B)], axis=0)
    return out.reshape(B, C, H, W)


# revision 87
# speedup vs baseline: 1.0039x; 1.0039x over previous
"""AttnBlock (GroupNorm -> QKV 1x1 convs -> full NxN attention -> out proj + residual)
for B=8, C=512, H=W=64 on 8 Trainium2 NeuronCores.

Sharding: data-parallel over batch, one sample per core. Each core holds the
full (host-pretransposed, bf16-cast) [C,C] conv weights and processes its
sample's [C, N=4096] activations entirely on-chip.

Per-core kernel outline (big matmuls fp8e4m3 DoubleRow with fp32 PSUM accumulate):
  1. GroupNorm(32 groups of 16 channels): bn_stats/bn_aggr per channel; since
     a group never straddles a 128-channel chunk, the group reduce/broadcast
     (tiny indicator matmuls on PE) and the ACT affine pass x*scl+bia -> h
     (bf16) pipeline per chunk.
  2. q = wq@h+bq, k = wk@h+bk as [C,N]; v computed directly transposed as
     vT = (h^T)@wv^T + bv -> [N,C] (so the attention V-matmul needs no
     transposes).
  3. Attention per 512-wide query block: S^T[j,i] = sum_c k[c,j] q[c,i]
     (accumulate 4 chunk matmuls in PSUM), P^T = exp(S^T/sqrt(C)) evicted by
     ScalarE to bf16 SBUF; softmax denominators by DVE-accumulating the 32
     P^T chunks then one ones-matmul partition reduce; O = V@P^T accumulated
     over the 32 j-chunks; normalize by broadcasting 1/denom across
     partitions with a K=1 matmul.
  4. out = wo@Ohat + bo + x (bias+residual fused in one DVE eviction).
"""

import numpy as np
import ml_dtypes

B = 8
C = 512
H = 64
W = 64
N = H * W            # 4096
P = 128
KC = C // P          # 4 channel chunks
NB = N // 512        # 8 blocks of 512
NT = N // P          # 32 tiles of 128
NGL = P // 16        # 8 groups per 128-channel chunk (group size 16)
EPS = 1e-6
SCALE = float(C) ** -0.5

_CACHE = {}

# Attention matmuls (S^T = K^T@Q and O = V@P^T) in fp8e4m3 with DoubleRow
# (K=256 per matmul, 2x PE throughput). exp outputs are pre-scaled by
# exp(PBIAS) to stay under e4m3's +-240 range; the softmax denominator gets
# the same factor, so normalization cancels it exactly.
FP8_ATTN = True
PBIAS = -2.772588722239781  # ln(1/16)
# QKV convs in fp8 DoubleRow, consuming an fp8 copy of x (GroupNorm stats also
# come from it: quantization noise averages out over the 65536-element groups).
FP8_CONV = True
# Output projection in fp8 DoubleRow.
FP8_WO = True
# Power-of-2 gains that keep fp8 operands out of the subnormal range (all are
# compensated exactly elsewhere):
#   GW: q/k/v weights scaled on device (w*scl*GW); q,k stored *GW so scores
#       come out *GW^2 -> exp scale divides by GW^2. v stored *GW -> folded
#       into the reciprocal row.
#   OH_GAIN: Ohat stored *OH_GAIN (fp8) -> final eviction divides it out.
#   GO: wo scaled *GO on the host -> final eviction divides it out.
GW = 8.0
OH_GAIN = 16.0
GO = 8.0
# ScalarE's exp stream paces the attention phase; offload these j-tiles (per
# 512-query block) to VectorE via the Schraudolph exp2 bit-trick:
#   exp(y) ~= bitcast_f32(int32((y*log2e + 127 - 0.043) * 2^23))
# One fused DVE multiply-add (fp32->int32) + one DVE copy (bitcast->fp8);
# ~3% relative error on ~12% of the attention weights.
EXP2_JTS = (11, 15, 19, 23, 27)  # mid-late: DVE runs the prior block's tail early on
LOG2E = 1.4426950408889634
EXP2_C = 0.04367744890362246


def _build_nc():
    import concourse.bass as bass
    import concourse.mybir as mybir
    import concourse.tile as tile
    from concourse import bacc

    fp32 = mybir.dt.float32
    bf16 = mybir.dt.bfloat16
    fp8 = mybir.dt.float8e4
    adt = fp8 if FP8_ATTN else bf16
    DR = mybir.MatmulPerfMode.DoubleRow
    AF = mybir.ActivationFunctionType
    Alu = mybir.AluOpType

    nc = bacc.Bacc("TRN2", target_bir_lowering=False, debug=False)

    cdt = fp8 if FP8_CONV else bf16  # conv activation dtype
    x_d = nc.dram_tensor("x", [C, N], fp32, kind="ExternalInput")
    xb_d = nc.dram_tensor("xb", [C, N], cdt, kind="ExternalInput")
    # Pretransposed conv weights: q, k, v in one tensor; wo separate (it is
    # not needed until phase B, so its DMA goes last).
    ww_d = nc.dram_tensor("ww", [3, C, C], bf16, kind="ExternalInput")
    wo_d = nc.dram_tensor("wod", [C, C], fp8 if FP8_WO else bf16, kind="ExternalInput")
    # Per-channel params packed: gn_scale, gn_bias, bq, bk, bo.
    prm_d = nc.dram_tensor("prm", [5, C], fp32, kind="ExternalInput")
    bv_d = nc.dram_tensor("bv", [C], fp32, kind="ExternalInput")
    ind_d = nc.dram_tensor("ind", [P, NGL], fp32, kind="ExternalInput")
    indT_d = nc.dram_tensor("indT", [NGL, P], fp32, kind="ExternalInput")
    out_d = nc.dram_tensor("out", [C, N], fp32, kind="ExternalOutput")

    with tile.TileContext(nc) as tc:
        with tc.tile_pool(name="persist", bufs=1) as pp:
            # ---------------- Phase A: GroupNorm + QKV convs ----------------
            with tc.tile_pool(name="pa", bufs=2) as pa, \
                 tc.tile_pool(name="pa_ps", bufs=1, space="PSUM") as pa_ps, \
                 tc.tile_pool(name="pa1", bufs=1) as pa1:
                # x (bf16) first: its DMAs gate the whole startup critical path.
                # Half-chunk transfers so bn_stats can start sooner.
                xbs = pa1.tile([P, KC, N], cdt, tag="xbs")
                nc.sync.dma_start(
                    out=xbs[:, 0, 0:2048], in_=xb_d.ap()[0:P, 0:2048]
                )
                # Small tensors needed by the early stats chain go ahead of the
                # bulk transfers.
                ind = pp.tile([P, NGL], fp32, tag="ind")
                nc.sync.dma_start(out=ind, in_=ind_d.ap())
                indT = pp.tile([NGL, P], fp32, tag="indT")
                nc.sync.dma_start(out=indT, in_=indT_d.ap())
                # Per-channel params in [p, kc] layout (channel = kc*128 + p).
                prm = pp.tile([P, 5, KC], fp32, tag="prm")
                nc.sync.dma_start(
                    out=prm, in_=prm_d.ap().rearrange("s (k p) -> p s k", p=P)
                )
                gns, gnb, bqs, bks, bos = (prm[:, i] for i in range(5))
                nc.sync.dma_start(
                    out=xbs[:, 0, 2048:4096], in_=xb_d.ap()[0:P, 2048:4096]
                )
                # Weights: [c', kc-of-c', c], contraction dim c' on partitions.
                ww = pp.tile([P, 3, KC, C], bf16, tag="ww")
                nc.sync.dma_start(
                    out=ww, in_=ww_d.ap().rearrange("w (kc p) c -> p w kc c", p=P)
                )
                wqt, wkt, wvt = (ww[:, i] for i in range(3))
                for kc in range(1, KC):
                    for hh in range(2):
                        nc.sync.dma_start(
                            out=xbs[:, kc, hh * 2048:(hh + 1) * 2048],
                            in_=xb_d.ap()[kc * P:(kc + 1) * P, hh * 2048:(hh + 1) * 2048],
                        )
                wot = pp.tile([P, KC, C], fp8 if FP8_WO else bf16, tag="wot")
                nc.sync.dma_start(
                    out=wot, in_=wo_d.ap().rearrange("(kc p) c -> p kc c", p=P)
                )
                bvr = pp.tile([1, C], fp32, tag="bvr")
                nc.sync.dma_start(
                    out=bvr, in_=bass.AP(tensor=bv_d, offset=0, ap=[[0, 1], [1, C]])
                )

                ones_f = pp.tile([P, 1], fp32, tag="ones_f")
                nc.vector.memset(ones_f, 1.0)
                ones_r = pp.tile([1, P], bf16, tag="ones_r")
                nc.vector.memset(ones_r, 1.0)
                eps8 = pp.tile([NGL, 1], fp32, tag="eps8")
                nc.vector.memset(eps8, EPS)
                pbias = pp.tile([P, 1], fp32, tag="pbias")
                nc.vector.memset(pbias, PBIAS)
                # DoubleRow lhsT needs the pair stride to be 16B-aligned, so
                # pad the ones column out to 16 elements and slice.
                ones8w = pp.tile([P, 2, 16], fp8, tag="ones8")
                nc.vector.memset(ones8w, 1.0)
                ones8 = ones8w[:, :, 0:1]

                qsb = pp.tile([P, KC, N], adt, tag="qsb")
                ksb = pp.tile([P, KC, N], adt, tag="ksb")
                vt = pp.tile([P, NT, C], adt, tag="vt")

                scl = pa1.tile([P, KC], fp32, tag="scl")
                sclG = pa1.tile([P, KC], fp32, tag="sclG")
                bia = pa1.tile([P, KC], fp32, tag="bia")
                bia_bf = pa1.tile([P, KC], bf16, tag="bia_bf")
                # GroupNorm affine folded into the conv weights:
                #   conv(w, h) = conv(w*scl, x) + w@bia, so scale each weight
                #   column c' by scl[c'] (per-partition multiply on the
                #   pretransposed weights) and fix the biases with tiny
                #   w@bia matmuls. x never needs normalizing, so PE can start
                #   on partial conv accumulations while later chunks' stats
                #   are still in flight.
                ws = pa1.tile([P, 3, KC, C], cdt, tag="ws")

                # Groups (16 channels) never straddle a 128-channel chunk, so
                # the whole stats -> scale/bias -> h chain pipelines per chunk.
                # Chunk 1's sums come from ScalarE's accumulate path (in-place
                # identity for sum(x), Square into a scratch for sum(x^2)) so
                # the DVE bn_stats chain that gates chunk 3's scale shortens.
                for kc in range(KC):
                    stats = pa.tile([P, 3], fp32, tag="stats")  # mean, var, mean^2
                    if kc == 1:
                        s12 = pa.tile([P, 2], fp32, tag="s12")
                        nc.scalar.activation(
                            out=xbs[:, kc, :], in_=xbs[:, kc, :], func=AF.Identity,
                            accum_out=s12[:, 0:1],
                        )
                        trash = pa.tile([P, N], cdt, tag="trash")
                        nc.scalar.activation(
                            out=trash, in_=xbs[:, kc, :], func=AF.Square,
                            accum_out=s12[:, 1:2],
                        )
                        nc.vector.tensor_scalar_mul(
                            out=stats[:, 0:1], in0=s12[:, 0:1], scalar1=1.0 / N
                        )
                        nc.vector.tensor_mul(
                            out=stats[:, 2:3], in0=stats[:, 0:1], in1=stats[:, 0:1]
                        )
                        nc.vector.tensor_scalar(
                            out=stats[:, 1:2], in0=s12[:, 1:2],
                            scalar1=1.0 / N, scalar2=stats[:, 2:3],
                            op0=Alu.mult, op1=Alu.subtract,
                        )
                    else:
                        st = pa.tile([P, 8, 6], fp32, tag="bnst")
                        for sg in range(8):
                            nc.vector.bn_stats(
                                out=st[:, sg, :], in_=xbs[:, kc, sg * 512:(sg + 1) * 512]
                            )
                        nc.vector.bn_aggr(out=stats[:, 0:2], in_=st)
                        nc.vector.tensor_mul(
                            out=stats[:, 2:3], in0=stats[:, 0:1], in1=stats[:, 0:1]
                        )
                    # Sum stats over the 16 channels of each group.
                    ps_st = pa_ps.tile([NGL, 3], fp32, tag="ps_st")
                    nc.tensor.matmul(ps_st, lhsT=ind, rhs=stats, start=True, stop=True)
                    st_s = pa.tile([NGL, 3], fp32, tag="st_s")
                    nc.vector.tensor_copy(out=st_s, in_=ps_st)

                    bcin = pa.tile([NGL, 2], fp32, tag="bcin")  # [:,0]=gmean [:,1]=rstd
                    nc.vector.tensor_scalar_mul(
                        out=bcin[:, 0:1], in0=st_s[:, 0:1], scalar1=1.0 / 16
                    )
                    gv = pa.tile([NGL, 1], fp32, tag="gv")
                    nc.vector.tensor_add(out=gv, in0=st_s[:, 1:2], in1=st_s[:, 2:3])
                    nc.vector.tensor_scalar_mul(out=gv, in0=gv, scalar1=1.0 / 16)
                    gm2 = pa.tile([NGL, 1], fp32, tag="gm2")
                    nc.vector.tensor_mul(out=gm2, in0=bcin[:, 0:1], in1=bcin[:, 0:1])
                    nc.vector.tensor_sub(out=gv, in0=gv, in1=gm2)  # var
                    nc.scalar.activation(out=gv, in_=gv, func=AF.Sqrt, bias=eps8)
                    nc.vector.reciprocal(out=bcin[:, 1:2], in_=gv)

                    # Broadcast group mean/rstd back to the chunk's channels.
                    ps_bc = pa_ps.tile([P, 2], fp32, tag="ps_bc")
                    nc.tensor.matmul(ps_bc, lhsT=indT, rhs=bcin, start=True, stop=True)
                    nc.vector.tensor_mul(
                        out=scl[:, kc:kc + 1], in0=ps_bc[:, 1:2], in1=gns[:, kc:kc + 1]
                    )
                    nc.vector.tensor_mul(
                        out=bia[:, kc:kc + 1], in0=ps_bc[:, 0:1], in1=scl[:, kc:kc + 1]
                    )
                    nc.vector.tensor_sub(
                        out=bia[:, kc:kc + 1], in0=gnb[:, kc:kc + 1], in1=bia[:, kc:kc + 1]
                    )
                    nc.vector.tensor_copy(
                        out=bia_bf[:, kc:kc + 1], in_=bia[:, kc:kc + 1]
                    )
                    # Weight scaling as a scaled ACT copy (keeps DVE free for
                    # the stats + vT eviction stream).
                    nc.vector.tensor_scalar_mul(
                        out=sclG[:, kc:kc + 1], in0=scl[:, kc:kc + 1],
                        scalar1=GW if FP8_CONV else 1.0,
                    )
                    for w in range(3):
                        nc.scalar.activation(
                            out=ws[:, w, kc, :], in_=ww[:, w, kc, :],
                            func=AF.Copy, scale=sclG[:, kc:kc + 1],
                        )

                # Folded conv biases: qb/kb[c] = w@bia + b  (per-partition in
                # the [C, N] output layout).
                qkb = pa1.tile([P, 2, KC], fp32, tag="qkb")
                for w, bsl in ((0, bqs), (1, bks)):
                    for ct in range(KC):
                        pqb = pa_ps.tile([P, 1], fp32, tag="qb", bufs=1)
                        for kc in range(KC):
                            nc.tensor.matmul(
                                pqb,
                                lhsT=ww[:, w, kc, ct * P:(ct + 1) * P],
                                rhs=bia_bf[:, kc:kc + 1],
                                start=(kc == 0), stop=(kc == KC - 1),
                            )
                        nc.vector.tensor_scalar(
                            out=qkb[:, w, ct:ct + 1], in0=pqb,
                            scalar1=bsl[:, ct:ct + 1],
                            scalar2=GW if FP8_CONV else 1.0,
                            op0=Alu.add, op1=Alu.mult,
                        )
                # vT bias row: vb[c] = bia@wvT + bv, broadcast to all partitions.
                pvb = pa_ps.tile([1, C], fp32, tag="vb", bufs=1)
                for kc in range(KC):
                    nc.tensor.matmul(
                        pvb, lhsT=bia_bf[:, kc:kc + 1], rhs=wvt[:, kc, :],
                        start=(kc == 0), stop=(kc == KC - 1),
                    )
                vbr = pa1.tile([1, C], fp32, tag="vbr")
                nc.vector.tensor_add(out=vbr, in0=pvb, in1=bvr)
                vbr_bf = pa1.tile([1, C], bf16, tag="vbr_bf")
                nc.vector.tensor_scalar_mul(
                    out=vbr_bf, in0=vbr, scalar1=GW if FP8_CONV else 1.0
                )
                pbv = pa_ps.tile([P, C], fp32, tag="convp", bufs=4)
                nc.tensor.matmul(pbv, lhsT=ones_r, rhs=vbr_bf, start=True, stop=True)
                bvb = pa1.tile([P, C], fp32, tag="bvb")
                nc.scalar.copy(out=bvb, in_=pbv)

                # q, k convs: [C, N] layout, consuming raw x.
                def conv_mms(psum, w_lhsT, w_rhs):
                    if FP8_CONV:
                        for k2 in range(KC // 2):
                            kk = slice(2 * k2, 2 * k2 + 2)
                            nc.tensor.matmul(
                                psum, lhsT=w_lhsT(kk), rhs=w_rhs(kk),
                                start=(k2 == 0), stop=(k2 == KC // 2 - 1),
                                perf_mode=DR,
                            )
                    else:
                        for kc in range(KC):
                            kk = slice(kc, kc + 1)
                            nc.tensor.matmul(
                                psum,
                                lhsT=w_lhsT(kk)[:, 0], rhs=w_rhs(kk)[:, 0],
                                start=(kc == 0), stop=(kc == KC - 1),
                            )

                # Interleave q/k/vT convs (nb-outer) so the eviction work
                # streams on ACT and DVE concurrently; k evictions alternate
                # between the two to balance their loads.
                for nb in range(NB):
                    for ct in range(KC):
                        pq = pa_ps.tile([P, 512], fp32, tag="convp", bufs=4)
                        conv_mms(
                            pq,
                            lambda kk, ct=ct: ws[:, 0, kk, ct * P:(ct + 1) * P],
                            lambda kk, nb=nb: xbs[:, kk, nb * 512:(nb + 1) * 512],
                        )
                        nc.scalar.activation(
                            out=qsb[:, ct, nb * 512:(nb + 1) * 512], in_=pq,
                            func=AF.Identity, bias=qkb[:, 0, ct:ct + 1],
                        )
                        pk = pa_ps.tile([P, 512], fp32, tag="convp", bufs=4)
                        conv_mms(
                            pk,
                            lambda kk, ct=ct: ws[:, 1, kk, ct * P:(ct + 1) * P],
                            lambda kk, nb=nb: xbs[:, kk, nb * 512:(nb + 1) * 512],
                        )
                        if ct % 4 != 3:
                            nc.scalar.activation(
                                out=ksb[:, ct, nb * 512:(nb + 1) * 512], in_=pk,
                                func=AF.Identity, bias=qkb[:, 1, ct:ct + 1],
                            )
                        else:
                            nc.vector.tensor_scalar_add(
                                out=ksb[:, ct, nb * 512:(nb + 1) * 512], in0=pk,
                                scalar1=qkb[:, 1, ct:ct + 1],
                            )
                    # vT conv: vT[n, c] = sum_c' x[c', n] ws_v[c', c] + vb[c].
                    for jt in range(4 * nb, 4 * nb + 4):
                        pv = pa_ps.tile([P, 512], fp32, tag="convp", bufs=4)
                        conv_mms(
                            pv,
                            lambda kk, jt=jt: xbs[:, kk, jt * P:(jt + 1) * P],
                            lambda kk: ws[:, 2, kk, :],
                        )
                        nc.vector.tensor_add(out=vt[:, jt, :], in0=pv, in1=bvb)

            # ---------------- Phase B: attention + output ----------------
            # Software-pipelined across query blocks: block ib-1's normalize /
            # projection tail is emitted inside block ib's score stream (the
            # broadcast matmul at jt==2, the projections at jt==8) so the
            # in-order PE always has S-matmuls to run while DVE works through
            # the reciprocal/normalize chain.
            with tc.tile_pool(name="pb", bufs=1) as pb, \
                 tc.tile_pool(name="pb_ps", bufs=1, space="PSUM") as pb_ps:
                escale = SCALE / (GW * GW) if FP8_CONV else SCALE
                rgain = (OH_GAIN if FP8_WO else 1.0) / (GW if FP8_CONV else 1.0)

                def tail1(ib, po, pd):
                    # 1/denom -> broadcast across partitions -> Ohat.
                    rc = pb.tile([1, 512], fp32, tag="rc", bufs=2)
                    nc.vector.reciprocal(out=rc, in_=pd)
                    rcb = pb.tile([1, 512], bf16, tag="rcb", bufs=2)
                    nc.vector.tensor_scalar_mul(out=rcb, in0=rc, scalar1=rgain)
                    pw = pb_ps.tile([P, 512], fp32, tag="w", bufs=1, name="pbc")
                    nc.tensor.matmul(pw, lhsT=ones_r, rhs=rcb, start=True, stop=True)
                    bcs = pb.tile([P, 512], fp32, tag="bcs", bufs=2)
                    nc.vector.tensor_copy(out=bcs, in_=pw)
                    oh = pb.tile([P, KC, 512], fp8 if FP8_WO else bf16, tag="oh", bufs=2)
                    for ct in range(KC):
                        nc.vector.tensor_mul(out=oh[:, ct, :], in0=po[ct], in1=bcs)
                    return oh

                def tail2(ib, oh, last=False):
                    # Output projection + bias + residual + store. The final
                    # block's projections use the freed 4-bank "o" tag so the
                    # four c-tiles retire in parallel instead of serializing
                    # on the single shared "w" bank.
                    for ct in range(KC):
                        if last:
                            pf = pb_ps.tile([P, 512], fp32, tag="o", bufs=4, name="pf")
                        else:
                            pf = pb_ps.tile([P, 512], fp32, tag="w", bufs=1, name="pf")
                        if FP8_WO:
                            for k2 in range(KC // 2):
                                nc.tensor.matmul(
                                    pf,
                                    lhsT=wot[:, 2 * k2:2 * k2 + 2, ct * P:(ct + 1) * P],
                                    rhs=oh[:, 2 * k2:2 * k2 + 2, :],
                                    start=(k2 == 0), stop=(k2 == KC // 2 - 1),
                                    perf_mode=DR,
                                )
                        else:
                            for kc in range(KC):
                                nc.tensor.matmul(
                                    pf,
                                    lhsT=wot[:, kc, ct * P:(ct + 1) * P],
                                    rhs=oh[:, kc, :],
                                    start=(kc == 0), stop=(kc == KC - 1),
                                )
                        xr = pb.tile([P, 512], fp32, tag="xr", bufs=3)
                        nc.sync.dma_start(
                            out=xr,
                            in_=x_d.ap()[ct * P:(ct + 1) * P, ib * 512:(ib + 1) * 512],
                        )
                        ob = pb.tile([P, 512], fp32, tag="ob", bufs=3)
                        if FP8_WO:
                            # x + bo staged on the idle GpSimd engine, then the
                            # eviction divides out OH_GAIN*GO.
                            xrb = pb.tile([P, 512], fp32, tag="xrb", bufs=3)
                            nc.gpsimd.tensor_scalar_add(
                                out=xrb, in0=xr, scalar1=bos[:, ct:ct + 1]
                            )
                            nc.vector.scalar_tensor_tensor(
                                out=ob, in0=pf, scalar=1.0 / (OH_GAIN * GO), in1=xrb,
                                op0=Alu.mult, op1=Alu.add,
                            )
                        else:
                            nc.vector.scalar_tensor_tensor(
                                out=ob, in0=pf, scalar=bos[:, ct:ct + 1], in1=xr,
                                op0=Alu.add, op1=Alu.add,
                            )
                        nc.sync.dma_start(
                            out=out_d.ap()[ct * P:(ct + 1) * P, ib * 512:(ib + 1) * 512],
                            in_=ob,
                        )

                pending = None  # (ib, po, pd) awaiting its tail
                pending_oh = None
                PEEL = 5

                def emit_sjt(ib, jt, pt):
                    ps = pb_ps.tile([P, 512], fp32, tag="s", bufs=3)
                    for k2 in range(KC // 2):
                        nc.tensor.matmul(
                            ps,
                            lhsT=ksb[:, 2 * k2:2 * k2 + 2, jt * P:(jt + 1) * P],
                            rhs=qsb[:, 2 * k2:2 * k2 + 2, ib * 512:(ib + 1) * 512],
                            start=(k2 == 0), stop=(k2 == KC // 2 - 1),
                            perf_mode=DR,
                        )
                    if jt in EXP2_JTS:
                        zi = pb.tile([P, 512], mybir.dt.int32, tag="zi", bufs=2)
                        nc.vector.tensor_scalar(
                            out=zi, in0=ps,
                            scalar1=escale * LOG2E * 8388608.0,
                            scalar2=(PBIAS * LOG2E + 127.0 - EXP2_C) * 8388608.0,
                            op0=Alu.mult, op1=Alu.add,
                        )
                        nc.vector.tensor_copy(out=pt[:, jt, :], in_=zi.bitcast(fp32))
                    else:
                        nc.scalar.activation(
                            out=pt[:, jt, :], in_=ps, func=AF.Exp,
                            scale=escale, bias=pbias,
                        )

                def new_pt():
                    pt = pb.tile([P, NT, 512], adt, tag="pt", bufs=2)
                    return pt

                # Peel the first PEEL score-tiles of each block ahead of the
                # previous block's O-accumulation so ScalarE keeps streaming
                # exp work across block boundaries.
                pt_cur = new_pt()
                for jt in range(PEEL):
                    emit_sjt(0, jt, pt_cur)
                for ib in range(NB):
                    for jt in range(PEEL, NT):
                        emit_sjt(ib, jt, pt_cur)
                        if jt == PEEL + 1 and pending is not None:
                            pending_oh = tail1(*pending)
                        if jt == 9 and pending is not None:
                            tail2(pending[0], pending_oh)
                            pending = pending_oh = None
                    pt_prev = pt_cur
                    if ib + 1 < NB:
                        pt_cur = new_pt()
                        for jt in range(PEEL):
                            emit_sjt(ib + 1, jt, pt_cur)
                    # Softmax denominators: column sums of P^T.
                    pd = pb_ps.tile([1, 512], fp32, tag="w", bufs=1, name="pd")
                    for jt2 in range(NT // 2):
                        nc.tensor.matmul(
                            pd, lhsT=ones8, rhs=pt_prev[:, 2 * jt2:2 * jt2 + 2, :],
                            start=(jt2 == 0), stop=(jt2 == NT // 2 - 1),
                            perf_mode=DR,
                        )
                    # O = V @ P^T, accumulated over j chunks.
                    po = [
                        pb_ps.tile([P, 512], fp32, tag="o", bufs=4, name=f"po{ct}")
                        for ct in range(KC)
                    ]
                    for jt2 in range(NT // 2):
                        for ct in range(KC):
                            nc.tensor.matmul(
                                po[ct],
                                lhsT=vt[:, 2 * jt2:2 * jt2 + 2, ct * P:(ct + 1) * P],
                                rhs=pt_prev[:, 2 * jt2:2 * jt2 + 2, :],
                                start=(jt2 == 0), stop=(jt2 == NT // 2 - 1),
                                perf_mode=DR,
                            )
                    pending = (ib, po, pd)
                # Last block's tail.
                oh = tail1(*pending)
                tail2(pending[0], oh, last=True)

    nc.compile()
    return nc


def _get_nc():
    if "nc" not in _CACHE:
        _CACHE["nc"] = _build_nc()
    return _CACHE["nc"]


def _indicator():
    ind = np.zeros((P, NGL), np.float32)
    for g in range(NGL):
        ind[g * 16:(g + 1) * 16, g] = 1.0
    return ind


def host_inputs(x, gn_scale, gn_bias, wq, bq, wk, bk, wv, bv, wo, bo):
    """Host-side reformatting: per-core input maps (x is [B?, C, H*W] or [C, H*W])."""
    bf = ml_dtypes.bfloat16
    f8 = ml_dtypes.float8_e4m3
    xf = np.ascontiguousarray(np.asarray(x, dtype=np.float32).reshape(-1, C, N))
    xbf = xf.astype(f8 if FP8_CONV else bf)
    ww = np.stack(
        [np.asarray(w, np.float32).T for w in (wq, wk, wv)], axis=0
    ).astype(bf)
    wod = np.ascontiguousarray(np.asarray(wo, np.float32).T * (GO if FP8_WO else 1.0)
                               ).astype(f8 if FP8_WO else bf)
    prm = np.stack(
        [np.asarray(v, np.float32) for v in (gn_scale, gn_bias, bq, bk, bo)], axis=0
    )
    common = {
        "ww": np.ascontiguousarray(ww),
        "wod": wod,
        "prm": np.ascontiguousarray(prm),
        "bv": np.asarray(bv, np.float32),
        "ind": _indicator(),
        "indT": np.ascontiguousarray(_indicator().T),
    }
    return [dict(common, x=xf[i], xb=xbf[i]) for i in range(xf.shape[0])]


def kernel(x, gn_scale, gn_bias, wq, bq, wk, bk, wv, bv, wo, bo):
    from concourse.bass_utils import run_bass_kernel_spmd

    nc = _get_nc()
    in_maps = host_inputs(x, gn_scale, gn_bias, wq, bq, wk, bk, wv, bv, wo, bo)
    res = run_bass_kernel_spmd(nc, in_maps, core_ids=list(range(B)))
    out = np.stack([res.results[i]["out"] for i in range(B)], axis=0)
    return out.reshape(B, C, H, W)


# revision 88
# speedup vs baseline: 1.0086x; 1.0046x over previous
"""AttnBlock (GroupNorm -> QKV 1x1 convs -> full NxN attention -> out proj + residual)
for B=8, C=512, H=W=64 on 8 Trainium2 NeuronCores.

Sharding: data-parallel over batch, one sample per core. Each core holds the
full (host-pretransposed, bf16-cast) [C,C] conv weights and processes its
sample's [C, N=4096] activations entirely on-chip.

Per-core kernel outline (big matmuls fp8e4m3 DoubleRow with fp32 PSUM accumulate):
  1. GroupNorm(32 groups of 16 channels): bn_stats/bn_aggr per channel; since
     a group never straddles a 128-channel chunk, the group reduce/broadcast
     (tiny indicator matmuls on PE) and the ACT affine pass x*scl+bia -> h
     (bf16) pipeline per chunk.
  2. q = wq@h+bq, k = wk@h+bk as [C,N]; v computed directly transposed as
     vT = (h^T)@wv^T + bv -> [N,C] (so the attention V-matmul needs no
     transposes).
  3. Attention per 512-wide query block: S^T[j,i] = sum_c k[c,j] q[c,i]
     (accumulate 4 chunk matmuls in PSUM), P^T = exp(S^T/sqrt(C)) evicted by
     ScalarE to bf16 SBUF; softmax denominators by DVE-accumulating the 32
     P^T chunks then one ones-matmul partition reduce; O = V@P^T accumulated
     over the 32 j-chunks; normalize by broadcasting 1/denom across
     partitions with a K=1 matmul.
  4. out = wo@Ohat + bo + x (bias+residual fused in one DVE eviction).
"""

import numpy as np
import ml_dtypes

B = 8
C = 512
H = 64
W = 64
N = H * W            # 4096
P = 128
KC = C // P          # 4 channel chunks
NB = N // 512        # 8 blocks of 512
NT = N // P          # 32 tiles of 128
NGL = P // 16        # 8 groups per 128-channel chunk (group size 16)
EPS = 1e-6
SCALE = float(C) ** -0.5

_CACHE = {}

# Attention matmuls (S^T = K^T@Q and O = V@P^T) in fp8e4m3 with DoubleRow
# (K=256 per matmul, 2x PE throughput). exp outputs are pre-scaled by
# exp(PBIAS) to stay under e4m3's +-240 range; the softmax denominator gets
# the same factor, so normalization cancels it exactly.
FP8_ATTN = True
PBIAS = -2.772588722239781  # ln(1/16)
# QKV convs in fp8 DoubleRow, consuming an fp8 copy of x (GroupNorm stats also
# come from it: quantization noise averages out over the 65536-element groups).
FP8_CONV = True
# Output projection in fp8 DoubleRow.
FP8_WO = True
# Power-of-2 gains that keep fp8 operands out of the subnormal range (all are
# compensated exactly elsewhere):
#   GW: q/k/v weights scaled on device (w*scl*GW); q,k stored *GW so scores
#       come out *GW^2 -> exp scale divides by GW^2. v stored *GW -> folded
#       into the reciprocal row.
#   OH_GAIN: Ohat stored *OH_GAIN (fp8) -> final eviction divides it out.
#   GO: wo scaled *GO on the host -> final eviction divides it out.
GW = 8.0
OH_GAIN = 16.0
GO = 8.0
# ScalarE's exp stream paces the attention phase; offload these j-tiles (per
# 512-query block) to VectorE via the Schraudolph exp2 bit-trick:
#   exp(y) ~= bitcast_f32(int32((y*log2e + 127 - 0.043) * 2^23))
# One fused DVE multiply-add (fp32->int32) + one DVE copy (bitcast->fp8);
# ~3% relative error on ~12% of the attention weights.
EXP2_JTS = (11, 15, 19, 23, 27)  # mid-late: DVE runs the prior block's tail early on
LOG2E = 1.4426950408889634
EXP2_C = 0.04367744890362246


def _build_nc():
    import concourse.bass as bass
    import concourse.mybir as mybir
    import concourse.tile as tile
    from concourse import bacc

    fp32 = mybir.dt.float32
    bf16 = mybir.dt.bfloat16
    fp8 = mybir.dt.float8e4
    adt = fp8 if FP8_ATTN else bf16
    DR = mybir.MatmulPerfMode.DoubleRow
    AF = mybir.ActivationFunctionType
    Alu = mybir.AluOpType

    nc = bacc.Bacc("TRN2", target_bir_lowering=False, debug=False)

    cdt = fp8 if FP8_CONV else bf16  # conv activation dtype
    x_d = nc.dram_tensor("x", [C, N], fp32, kind="ExternalInput")
    xb_d = nc.dram_tensor("xb", [C, N], cdt, kind="ExternalInput")
    # Pretransposed conv weights: q, k, v in one tensor; wo separate (it is
    # not needed until phase B, so its DMA goes last).
    ww_d = nc.dram_tensor("ww", [3, C, C], bf16, kind="ExternalInput")
    wo_d = nc.dram_tensor("wod", [C, C], fp8 if FP8_WO else bf16, kind="ExternalInput")
    # Per-channel params packed: gn_scale, gn_bias, bq, bk, bo.
    prm_d = nc.dram_tensor("prm", [5, C], fp32, kind="ExternalInput")
    bv_d = nc.dram_tensor("bv", [C], fp32, kind="ExternalInput")
    ind_d = nc.dram_tensor("ind", [P, NGL], fp32, kind="ExternalInput")
    indT_d = nc.dram_tensor("indT", [NGL, P], fp32, kind="ExternalInput")
    out_d = nc.dram_tensor("out", [C, N], fp32, kind="ExternalOutput")

    with tile.TileContext(nc) as tc:
        with tc.tile_pool(name="persist", bufs=1) as pp:
            # ---------------- Phase A: GroupNorm + QKV convs ----------------
            with tc.tile_pool(name="pa", bufs=2) as pa, \
                 tc.tile_pool(name="pa_ps", bufs=1, space="PSUM") as pa_ps, \
                 tc.tile_pool(name="pa1", bufs=1) as pa1:
                # x (bf16) first: its DMAs gate the whole startup critical path.
                # Half-chunk transfers so bn_stats can start sooner.
                xbs = pa1.tile([P, KC, N], cdt, tag="xbs")
                nc.sync.dma_start(
                    out=xbs[:, 0, 0:2048], in_=xb_d.ap()[0:P, 0:2048]
                )
                # Small tensors needed by the early stats chain go ahead of the
                # bulk transfers.
                ind = pp.tile([P, NGL], fp32, tag="ind")
                nc.sync.dma_start(out=ind, in_=ind_d.ap())
                indT = pp.tile([NGL, P], fp32, tag="indT")
                nc.sync.dma_start(out=indT, in_=indT_d.ap())
                # Per-channel params in [p, kc] layout (channel = kc*128 + p).
                prm = pp.tile([P, 5, KC], fp32, tag="prm")
                nc.sync.dma_start(
                    out=prm, in_=prm_d.ap().rearrange("s (k p) -> p s k", p=P)
                )
                gns, gnb, bqs, bks, bos = (prm[:, i] for i in range(5))
                nc.sync.dma_start(
                    out=xbs[:, 0, 2048:4096], in_=xb_d.ap()[0:P, 2048:4096]
                )
                # Weights: [c', kc-of-c', c], contraction dim c' on partitions.
                ww = pp.tile([P, 3, KC, C], bf16, tag="ww")
                nc.sync.dma_start(
                    out=ww, in_=ww_d.ap().rearrange("w (kc p) c -> p w kc c", p=P)
                )
                wqt, wkt, wvt = (ww[:, i] for i in range(3))
                for kc in range(1, KC):
                    for hh in range(2):
                        nc.sync.dma_start(
                            out=xbs[:, kc, hh * 2048:(hh + 1) * 2048],
                            in_=xb_d.ap()[kc * P:(kc + 1) * P, hh * 2048:(hh + 1) * 2048],
                        )
                wot = pp.tile([P, KC, C], fp8 if FP8_WO else bf16, tag="wot")
                nc.sync.dma_start(
                    out=wot, in_=wo_d.ap().rearrange("(kc p) c -> p kc c", p=P)
                )
                bvr = pp.tile([1, C], fp32, tag="bvr")
                nc.sync.dma_start(
                    out=bvr, in_=bass.AP(tensor=bv_d, offset=0, ap=[[0, 1], [1, C]])
                )

                ones_f = pp.tile([P, 1], fp32, tag="ones_f")
                nc.vector.memset(ones_f, 1.0)
                ones_r = pp.tile([1, P], bf16, tag="ones_r")
                nc.vector.memset(ones_r, 1.0)
                eps8 = pp.tile([NGL, 1], fp32, tag="eps8")
                nc.vector.memset(eps8, EPS)
                pbias = pp.tile([P, 1], fp32, tag="pbias")
                nc.vector.memset(pbias, PBIAS)
                # DoubleRow lhsT needs the pair stride to be 16B-aligned, so
                # pad the ones column out to 16 elements and slice.
                ones8w = pp.tile([P, 2, 16], fp8, tag="ones8")
                nc.vector.memset(ones8w, 1.0)
                ones8 = ones8w[:, :, 0:1]

                qsb = pp.tile([P, KC, N], adt, tag="qsb")
                ksb = pp.tile([P, KC, N], adt, tag="ksb")
                vt = pp.tile([P, NT, C], adt, tag="vt")

                scl = pa1.tile([P, KC], fp32, tag="scl")
                sclG = pa1.tile([P, KC], fp32, tag="sclG")
                bia = pa1.tile([P, KC], fp32, tag="bia")
                bia_bf = pa1.tile([P, KC], bf16, tag="bia_bf")
                # GroupNorm affine folded into the conv weights:
                #   conv(w, h) = conv(w*scl, x) + w@bia, so scale each weight
                #   column c' by scl[c'] (per-partition multiply on the
                #   pretransposed weights) and fix the biases with tiny
                #   w@bia matmuls. x never needs normalizing, so PE can start
                #   on partial conv accumulations while later chunks' stats
                #   are still in flight.
                ws = pa1.tile([P, 3, KC, C], cdt, tag="ws")

                # Groups (16 channels) never straddle a 128-channel chunk, so
                # the whole stats -> scale/bias -> h chain pipelines per chunk.
                # Chunk 1's sums come from ScalarE's accumulate path (in-place
                # identity for sum(x), Square into a scratch for sum(x^2)) so
                # the DVE bn_stats chain that gates chunk 3's scale shortens.
                for kc in range(KC):
                    stats = pa.tile([P, 3], fp32, tag="stats")  # mean, var, mean^2
                    if kc == 1:
                        s12 = pa.tile([P, 2], fp32, tag="s12")
                        nc.scalar.activation(
                            out=xbs[:, kc, :], in_=xbs[:, kc, :], func=AF.Identity,
                            accum_out=s12[:, 0:1],
                        )
                        trash = pa.tile([P, N], cdt, tag="trash")
                        nc.scalar.activation(
                            out=trash, in_=xbs[:, kc, :], func=AF.Square,
                            accum_out=s12[:, 1:2],
                        )
                        nc.vector.tensor_scalar_mul(
                            out=stats[:, 0:1], in0=s12[:, 0:1], scalar1=1.0 / N
                        )
                        nc.vector.tensor_mul(
                            out=stats[:, 2:3], in0=stats[:, 0:1], in1=stats[:, 0:1]
                        )
                        nc.vector.tensor_scalar(
                            out=stats[:, 1:2], in0=s12[:, 1:2],
                            scalar1=1.0 / N, scalar2=stats[:, 2:3],
                            op0=Alu.mult, op1=Alu.subtract,
                        )
                    else:
                        st = pa.tile([P, 8, 6], fp32, tag="bnst")
                        for sg in range(8):
                            nc.vector.bn_stats(
                                out=st[:, sg, :], in_=xbs[:, kc, sg * 512:(sg + 1) * 512]
                            )
                        nc.vector.bn_aggr(out=stats[:, 0:2], in_=st)
                        nc.vector.tensor_mul(
                            out=stats[:, 2:3], in0=stats[:, 0:1], in1=stats[:, 0:1]
                        )
                    # Sum stats over the 16 channels of each group.
                    ps_st = pa_ps.tile([NGL, 3], fp32, tag="ps_st")
                    nc.tensor.matmul(ps_st, lhsT=ind, rhs=stats, start=True, stop=True)
                    st_s = pa.tile([NGL, 3], fp32, tag="st_s")
                    nc.vector.tensor_copy(out=st_s, in_=ps_st)

                    bcin = pa.tile([NGL, 2], fp32, tag="bcin")  # [:,0]=gmean [:,1]=rstd
                    nc.vector.tensor_scalar_mul(
                        out=bcin[:, 0:1], in0=st_s[:, 0:1], scalar1=1.0 / 16
                    )
                    gv = pa.tile([NGL, 1], fp32, tag="gv")
                    nc.vector.tensor_add(out=gv, in0=st_s[:, 1:2], in1=st_s[:, 2:3])
                    nc.vector.tensor_scalar_mul(out=gv, in0=gv, scalar1=1.0 / 16)
                    gm2 = pa.tile([NGL, 1], fp32, tag="gm2")
                    nc.vector.tensor_mul(out=gm2, in0=bcin[:, 0:1], in1=bcin[:, 0:1])
                    nc.vector.tensor_sub(out=gv, in0=gv, in1=gm2)  # var
                    nc.scalar.activation(out=gv, in_=gv, func=AF.Sqrt, bias=eps8)
                    nc.vector.reciprocal(out=bcin[:, 1:2], in_=gv)

                    # Broadcast group mean/rstd back to the chunk's channels.
                    ps_bc = pa_ps.tile([P, 2], fp32, tag="ps_bc")
                    nc.tensor.matmul(ps_bc, lhsT=indT, rhs=bcin, start=True, stop=True)
                    nc.vector.tensor_mul(
                        out=scl[:, kc:kc + 1], in0=ps_bc[:, 1:2], in1=gns[:, kc:kc + 1]
                    )
                    nc.vector.tensor_mul(
                        out=bia[:, kc:kc + 1], in0=ps_bc[:, 0:1], in1=scl[:, kc:kc + 1]
                    )
                    nc.vector.tensor_sub(
                        out=bia[:, kc:kc + 1], in0=gnb[:, kc:kc + 1], in1=bia[:, kc:kc + 1]
                    )
                    nc.vector.tensor_copy(
                        out=bia_bf[:, kc:kc + 1], in_=bia[:, kc:kc + 1]
                    )
                    # Weight scaling on the idle GpSimd engine: keeps ACT's
                    # queue short so the tiny critical Sqrt/scale ops (which
                    # gate each chunk's convs) are not stuck behind it.
                    for w in range(3):
                        nc.gpsimd.tensor_scalar(
                            out=ws[:, w, kc, :], in0=ww[:, w, kc, :],
                            scalar1=scl[:, kc:kc + 1],
                            scalar2=GW if FP8_CONV else 1.0,
                            op0=Alu.mult, op1=Alu.mult,
                        )

                # Folded conv biases: qb/kb[c] = w@bia + b  (per-partition in
                # the [C, N] output layout).
                qkb = pa1.tile([P, 2, KC], fp32, tag="qkb")
                for w, bsl in ((0, bqs), (1, bks)):
                    for ct in range(KC):
                        pqb = pa_ps.tile([P, 1], fp32, tag="qb", bufs=1)
                        for kc in range(KC):
                            nc.tensor.matmul(
                                pqb,
                                lhsT=ww[:, w, kc, ct * P:(ct + 1) * P],
                                rhs=bia_bf[:, kc:kc + 1],
                                start=(kc == 0), stop=(kc == KC - 1),
                            )
                        nc.vector.tensor_scalar(
                            out=qkb[:, w, ct:ct + 1], in0=pqb,
                            scalar1=bsl[:, ct:ct + 1],
                            scalar2=GW if FP8_CONV else 1.0,
                            op0=Alu.add, op1=Alu.mult,
                        )
                # vT bias row: vb[c] = bia@wvT + bv, broadcast to all partitions.
                pvb = pa_ps.tile([1, C], fp32, tag="vb", bufs=1)
                for kc in range(KC):
                    nc.tensor.matmul(
                        pvb, lhsT=bia_bf[:, kc:kc + 1], rhs=wvt[:, kc, :],
                        start=(kc == 0), stop=(kc == KC - 1),
                    )
                vbr = pa1.tile([1, C], fp32, tag="vbr")
                nc.vector.tensor_add(out=vbr, in0=pvb, in1=bvr)
                vbr_bf = pa1.tile([1, C], bf16, tag="vbr_bf")
                nc.vector.tensor_scalar_mul(
                    out=vbr_bf, in0=vbr, scalar1=GW if FP8_CONV else 1.0
                )
                pbv = pa_ps.tile([P, C], fp32, tag="convp", bufs=4)
                nc.tensor.matmul(pbv, lhsT=ones_r, rhs=vbr_bf, start=True, stop=True)
                bvb = pa1.tile([P, C], fp32, tag="bvb")
                nc.scalar.copy(out=bvb, in_=pbv)

                # q, k convs: [C, N] layout, consuming raw x.
                def conv_mms(psum, w_lhsT, w_rhs):
                    if FP8_CONV:
                        for k2 in range(KC // 2):
                            kk = slice(2 * k2, 2 * k2 + 2)
                            nc.tensor.matmul(
                                psum, lhsT=w_lhsT(kk), rhs=w_rhs(kk),
                                start=(k2 == 0), stop=(k2 == KC // 2 - 1),
                                perf_mode=DR,
                            )
                    else:
                        for kc in range(KC):
                            kk = slice(kc, kc + 1)
                            nc.tensor.matmul(
                                psum,
                                lhsT=w_lhsT(kk)[:, 0], rhs=w_rhs(kk)[:, 0],
                                start=(kc == 0), stop=(kc == KC - 1),
                            )

                # Interleave q/k/vT convs (nb-outer) so the eviction work
                # streams on ACT and DVE concurrently; k evictions alternate
                # between the two to balance their loads.
                for nb in range(NB):
                    for ct in range(KC):
                        pq = pa_ps.tile([P, 512], fp32, tag="convp", bufs=4)
                        conv_mms(
                            pq,
                            lambda kk, ct=ct: ws[:, 0, kk, ct * P:(ct + 1) * P],
                            lambda kk, nb=nb: xbs[:, kk, nb * 512:(nb + 1) * 512],
                        )
                        nc.scalar.activation(
                            out=qsb[:, ct, nb * 512:(nb + 1) * 512], in_=pq,
                            func=AF.Identity, bias=qkb[:, 0, ct:ct + 1],
                        )
                        pk = pa_ps.tile([P, 512], fp32, tag="convp", bufs=4)
                        conv_mms(
                            pk,
                            lambda kk, ct=ct: ws[:, 1, kk, ct * P:(ct + 1) * P],
                            lambda kk, nb=nb: xbs[:, kk, nb * 512:(nb + 1) * 512],
                        )
                        if ct % 4 != 3:
                            nc.scalar.activation(
                                out=ksb[:, ct, nb * 512:(nb + 1) * 512], in_=pk,
                                func=AF.Identity, bias=qkb[:, 1, ct:ct + 1],
                            )
                        else:
                            nc.vector.tensor_scalar_add(
                                out=ksb[:, ct, nb * 512:(nb + 1) * 512], in0=pk,
                                scalar1=qkb[:, 1, ct:ct + 1],
                            )
                    # vT conv: vT[n, c] = sum_c' x[c', n] ws_v[c', c] + vb[c].
                    for jt in range(4 * nb, 4 * nb + 4):
                        pv = pa_ps.tile([P, 512], fp32, tag="convp", bufs=4)
                        conv_mms(
                            pv,
                            lambda kk, jt=jt: xbs[:, kk, jt * P:(jt + 1) * P],
                            lambda kk: ws[:, 2, kk, :],
                        )
                        nc.vector.tensor_add(out=vt[:, jt, :], in0=pv, in1=bvb)

            # ---------------- Phase B: attention + output ----------------
            # Software-pipelined across query blocks: block ib-1's normalize /
            # projection tail is emitted inside block ib's score stream (the
            # broadcast matmul at jt==2, the projections at jt==8) so the
            # in-order PE always has S-matmuls to run while DVE works through
            # the reciprocal/normalize chain.
            with tc.tile_pool(name="pb", bufs=1) as pb, \
                 tc.tile_pool(name="pb_ps", bufs=1, space="PSUM") as pb_ps:
                escale = SCALE / (GW * GW) if FP8_CONV else SCALE
                rgain = (OH_GAIN if FP8_WO else 1.0) / (GW if FP8_CONV else 1.0)

                def tail1(ib, po, pd):
                    # 1/denom -> broadcast across partitions -> Ohat.
                    rc = pb.tile([1, 512], fp32, tag="rc", bufs=2)
                    nc.vector.reciprocal(out=rc, in_=pd)
                    rcb = pb.tile([1, 512], bf16, tag="rcb", bufs=2)
                    nc.vector.tensor_scalar_mul(out=rcb, in0=rc, scalar1=rgain)
                    pw = pb_ps.tile([P, 512], fp32, tag="w", bufs=1, name="pbc")
                    nc.tensor.matmul(pw, lhsT=ones_r, rhs=rcb, start=True, stop=True)
                    bcs = pb.tile([P, 512], fp32, tag="bcs", bufs=2)
                    nc.vector.tensor_copy(out=bcs, in_=pw)
                    oh = pb.tile([P, KC, 512], fp8 if FP8_WO else bf16, tag="oh", bufs=2)
                    for ct in range(KC):
                        nc.vector.tensor_mul(out=oh[:, ct, :], in0=po[ct], in1=bcs)
                    return oh

                def tail2(ib, oh, last=False):
                    # Output projection + bias + residual + store. The final
                    # block's projections use the freed 4-bank "o" tag so the
                    # four c-tiles retire in parallel instead of serializing
                    # on the single shared "w" bank.
                    for ct in range(KC):
                        if last:
                            pf = pb_ps.tile([P, 512], fp32, tag="o", bufs=4, name="pf")
                        else:
                            pf = pb_ps.tile([P, 512], fp32, tag="w", bufs=1, name="pf")
                        if FP8_WO:
                            for k2 in range(KC // 2):
                                nc.tensor.matmul(
                                    pf,
                                    lhsT=wot[:, 2 * k2:2 * k2 + 2, ct * P:(ct + 1) * P],
                                    rhs=oh[:, 2 * k2:2 * k2 + 2, :],
                                    start=(k2 == 0), stop=(k2 == KC // 2 - 1),
                                    perf_mode=DR,
                                )
                        else:
                            for kc in range(KC):
                                nc.tensor.matmul(
                                    pf,
                                    lhsT=wot[:, kc, ct * P:(ct + 1) * P],
                                    rhs=oh[:, kc, :],
                                    start=(kc == 0), stop=(kc == KC - 1),
                                )
                        xr = pb.tile([P, 512], fp32, tag="xr", bufs=3)
                        nc.sync.dma_start(
                            out=xr,
                            in_=x_d.ap()[ct * P:(ct + 1) * P, ib * 512:(ib + 1) * 512],
                        )
                        ob = pb.tile([P, 512], fp32, tag="ob", bufs=3)
                        if FP8_WO:
                            # x + bo staged on the idle GpSimd engine, then the
                            # eviction divides out OH_GAIN*GO.
                            xrb = pb.tile([P, 512], fp32, tag="xrb", bufs=3)
                            nc.gpsimd.tensor_scalar_add(
                                out=xrb, in0=xr, scalar1=bos[:, ct:ct + 1]
                            )
                            nc.vector.scalar_tensor_tensor(
                                out=ob, in0=pf, scalar=1.0 / (OH_GAIN * GO), in1=xrb,
                                op0=Alu.mult, op1=Alu.add,
                            )
                        else:
                            nc.vector.scalar_tensor_tensor(
                                out=ob, in0=pf, scalar=bos[:, ct:ct + 1], in1=xr,
                                op0=Alu.add, op1=Alu.add,
                            )
                        nc.sync.dma_start(
                            out=out_d.ap()[ct * P:(ct + 1) * P, ib * 512:(ib + 1) * 512],
                            in_=ob,
                        )

                pending = None  # (ib, po, pd) awaiting its tail
                pending_oh = None
                PEEL = 5

                def emit_sjt(ib, jt, pt):
                    ps = pb_ps.tile([P, 512], fp32, tag="s", bufs=3)
                    for k2 in range(KC // 2):
                        nc.tensor.matmul(
                            ps,
                            lhsT=ksb[:, 2 * k2:2 * k2 + 2, jt * P:(jt + 1) * P],
                            rhs=qsb[:, 2 * k2:2 * k2 + 2, ib * 512:(ib + 1) * 512],
                            start=(k2 == 0), stop=(k2 == KC // 2 - 1),
                            perf_mode=DR,
                        )
                    if jt in EXP2_JTS:
                        zi = pb.tile([P, 512], mybir.dt.int32, tag="zi", bufs=2)
                        nc.vector.tensor_scalar(
                            out=zi, in0=ps,
                            scalar1=escale * LOG2E * 8388608.0,
                            scalar2=(PBIAS * LOG2E + 127.0 - EXP2_C) * 8388608.0,
                            op0=Alu.mult, op1=Alu.add,
                        )
                        nc.vector.tensor_copy(out=pt[:, jt, :], in_=zi.bitcast(fp32))
                    else:
                        nc.scalar.activation(
                            out=pt[:, jt, :], in_=ps, func=AF.Exp,
                            scale=escale, bias=pbias,
                        )

                def new_pt():
                    pt = pb.tile([P, NT, 512], adt, tag="pt", bufs=2)
                    return pt

                # Peel the first PEEL score-tiles of each block ahead of the
                # previous block's O-accumulation so ScalarE keeps streaming
                # exp work across block boundaries.
                pt_cur = new_pt()
                for jt in range(PEEL):
                    emit_sjt(0, jt, pt_cur)
                for ib in range(NB):
                    for jt in range(PEEL, NT):
                        emit_sjt(ib, jt, pt_cur)
                        if jt == PEEL + 1 and pending is not None:
                            pending_oh = tail1(*pending)
                        if jt == 9 and pending is not None:
                            tail2(pending[0], pending_oh)
                            pending = pending_oh = None
                    pt_prev = pt_cur
                    if ib + 1 < NB:
                        pt_cur = new_pt()
                        for jt in range(PEEL):
                            emit_sjt(ib + 1, jt, pt_cur)
                    # Softmax denominators: column sums of P^T.
                    pd = pb_ps.tile([1, 512], fp32, tag="w", bufs=1, name="pd")
                    for jt2 in range(NT // 2):
                        nc.tensor.matmul(
                            pd, lhsT=ones8, rhs=pt_prev[:, 2 * jt2:2 * jt2 + 2, :],
                            start=(jt2 == 0), stop=(jt2 == NT // 2 - 1),
                            perf_mode=DR,
                        )
                    # O = V @ P^T, accumulated over j chunks.
                    po = [
                        pb_ps.tile([P, 512], fp32, tag="o", bufs=4, name=f"po{ct}")
                        for ct in range(KC)
                    ]
                    for jt2 in range(NT // 2):
                        for ct in range(KC):
                            nc.tensor.matmul(
                                po[ct],
                                lhsT=vt[:, 2 * jt2:2 * jt2 + 2, ct * P:(ct + 1) * P],
                                rhs=pt_prev[:, 2 * jt2:2 * jt2 + 2, :],
                                start=(jt2 == 0), stop=(jt2 == NT // 2 - 1),
                                perf_mode=DR,
                            )
                    pending = (ib, po, pd)
                # Last block's tail.
                oh = tail1(*pending)
                tail2(pending[0], oh, last=True)

    nc.compile()
    return nc


def _get_nc():
    if "nc" not in _CACHE:
        _CACHE["nc"] = _build_nc()
    return _CACHE["nc"]


def _indicator():
    ind = np.zeros((P, NGL), np.float32)
    for g in range(NGL):
        ind[g * 16:(g + 1) * 16, g] = 1.0
    return ind


def host_inputs(x, gn_scale, gn_bias, wq, bq, wk, bk, wv, bv, wo, bo):
    """Host-side reformatting: per-core input maps (x is [B?, C, H*W] or [C, H*W])."""
    bf = ml_dtypes.bfloat16
    f8 = ml_dtypes.float8_e4m3
    xf = np.ascontiguousarray(np.asarray(x, dtype=np.float32).reshape(-1, C, N))
    xbf = xf.astype(f8 if FP8_CONV else bf)
    ww = np.stack(
        [np.asarray(w, np.float32).T for w in (wq, wk, wv)], axis=0
    ).astype(bf)
    wod = np.ascontiguousarray(np.asarray(wo, np.float32).T * (GO if FP8_WO else 1.0)
                               ).astype(f8 if FP8_WO else bf)
    prm = np.stack(
        [np.asarray(v, np.float32) for v in (gn_scale, gn_bias, bq, bk, bo)], axis=0
    )
    common = {
        "ww": np.ascontiguousarray(ww),
        "wod": wod,
        "prm": np.ascontiguousarray(prm),
        "bv": np.asarray(bv, np.float32),
        "ind": _indicator(),
        "indT": np.ascontiguousarray(_indicator().T),
    }
    return [dict(common, x=xf[i], xb=xbf[i]) for i in range(xf.shape[0])]


def kernel(x, gn_scale, gn_bias, wq, bq, wk, bk, wv, bv, wo, bo):
    from concourse.bass_utils import run_bass_kernel_spmd

    nc = _get_nc()
    in_maps = host_inputs(x, gn_scale, gn_bias, wq, bq, wk, bk, wv, bv, wo, bo)
    res = run_bass_kernel_spmd(nc, in_maps, core_ids=list(range(B)))
    out = np.stack([res.results[i]["out"] for i in range(B)], axis=0)
    return out.reshape(B, C, H, W)
